# revision 3
# baseline (speedup 1.0000x reference)
"""Dilated attention (LongNet-style) Trainium2 kernel — v3 (paired PV).

Problem: query/key/value (2, 8192, 12, 64) f32. Three dilation groups
(segment lengths 2048/4096/8192, dilation 1/2/4, head slices 0:4/4:8/8:12).
Each group's gather produces independent dense attention over 2048-position
dilated segments; outputs are normalized per (batch, head, channel) by the
sum over all segment positions, and divided by num_groups.

Sharding: 8 cores = 2 batches x 4 "head columns". Core c owns batch c//4 and
heads {j, 4+j, 8+j} where j = c%4 -- exactly 7 dense 2048x2048x64 attention
units per core (4 + 2 + 1 segments), perfectly balanced, with all segments of
any (batch, head) on one core so normalization needs no cross-core traffic.

Precision (validated by numpy simulation of the exact arithmetic): the
x / x.sum normalization amplifies V-path errors ~140x but score/P-path
errors only ~8-15x. So K keeps an fp16 hi/lo pair packed along the
contraction dim (khl rows 0-63 = kh, 64-127 = kl; qhh rows = qh duplicated,
so scores = (kh+kl)^T qh in ONE fp16 matmul), Q and P are single fp16, and
V keeps an fp16 hi/lo pair. Measured HW end-to-end: 5.9e-3 (threshold 2e-2).

v3 layout: PV output uses only 64 partitions (the V channels), so TWO
q-chunks' PV matmuls run concurrently on disjoint PE column groups
(tile_position (0,0) / (0,64)) — halving PV cost. The softmax denominator
(previously a 65th ones-column of vh) becomes M=1 matmuls against a ones
vector, also column-paired. Per (128 kpos x 2x512 q) step:
  2x score MM (M=128, serial) + 1 paired PV-vh + 1 paired PV-vl +
  1 paired den = 5 x 216ns PE vs ACT exp over [128,1024] ~1.1us. ~Balanced.

Device kernel (same program on all 8 cores, different data):
  - inputs: qhh/khl [128, 14336] fp16 (as above), v1h/v1l [128, 7168] fp16
    (256*V hi/lo per 128-row k-block, 64 channels, no ones column).
  - 14 chunk-pairs (A=2p, B=2p+1, same segment) x 16 k-blocks:
      S^T[:, :512]=khl_kb.T @ qhh_A; S^T[:, 512:]=khl_kb.T @ qhh_B (PSUM f32)
      p1 = fp16(exp(S^T*0.125/65536 + ln64))    (ACT, one pass, PSUM->SBUF)
      ot[0:64]   += vh_kb.T@p1_A + vl_kb.T@p1_A   (PE, col group 0-1)
      ot[64:128] += vh_kb.T@p1_B + vl_kb.T@p1_B   (PE, col group 2-3, concur)
      den[0:1] += ones.T@p1_A; den[64:65] += ones.T@p1_B  (M=1, col-paired)
  - ot/den copied PSUM->SBUF (DVE), streamed to DRAM out2 [128, 7168] /
    outd [2, 7168] f32.
Host divides num/den, applies the group normalization (sum over positions
per channel) and the /3, and scatters into the (2, 8192, 12, 64) output.
"""

import os
import sys

if "/opt/trn_rl_repo" not in sys.path:
    sys.path.insert(0, "/opt/trn_rl_repo")
if "jax" not in sys.modules:
    os.environ.setdefault("JAX_PLATFORMS", "axon")

import numpy as np

import concourse.bass as bass  # noqa: F401
import concourse.mybir as mybir
import concourse.tile as tile
from concourse import bacc
from concourse.bass_utils import run_bass_kernel_spmd

F32 = mybir.dt.float32
F16 = mybir.dt.float16

B, N, H, D = 2, 8192, 12, 64
NSEG = 7           # segments per core
SEG = 2048         # dilated segment length
NCHUNK = NSEG * 4  # 512-wide q chunks per core
NPAIR = NCHUNK // 2
NKB = 16           # 128-row k blocks per segment
NSTEP = NPAIR * NKB
QSC = np.float32(256.0)               # fp16 pre-scale for Q/K/V splits
ESC = float(0.125 / (256.0 * 256.0))  # exp scale: 1/sqrt(64) + descale
import math
PBIAS = float(math.log(64.0))         # exp bias: P *= 64, into fp16-normal range

_CACHE = {}
LAST_RESULT = {}


def _build_nc():
    nc = bacc.Bacc("TRN2", target_bir_lowering=False, debug=False,
                   enable_asserts=False, num_devices=8)
    qhh = nc.dram_tensor("qhh", [128, NSEG * SEG], F16, kind="ExternalInput")
    khl = nc.dram_tensor("khl", [128, NSEG * SEG], F16, kind="ExternalInput")
    v1h = nc.dram_tensor("v1h", [128, NSEG * NKB * 64], F16, kind="ExternalInput")
    v1l = nc.dram_tensor("v1l", [128, NSEG * NKB * 64], F16, kind="ExternalInput")
    out2 = nc.dram_tensor("out2", [128, NPAIR * 512], F32, kind="ExternalOutput")
    outd = nc.dram_tensor("outd", [2, NPAIR * 512], F32, kind="ExternalOutput")
    qhh_ap, khl_ap, v1h_ap, v1l_ap = (
        qhh.ap(), khl.ap(), v1h.ap(), v1l.ap())
    out2_ap, outd_ap = out2.ap(), outd.ap()

    with tile.TileContext(nc) as tc:
        with (
            tc.tile_pool(name="inp", bufs=1) as inp,
            tc.tile_pool(name="pt", bufs=4) as ptp,
            tc.tile_pool(name="osb", bufs=2) as osbp,
            tc.tile_pool(name="score", bufs=2, space="PSUM") as scp,
            tc.tile_pool(name="ot", bufs=2, space="PSUM") as otp,
            tc.tile_pool(name="den", bufs=2, space="PSUM") as denp,
        ):
            bias_t = inp.tile([128, 1], F32, tag="bias", name="bias_t")
            nc.vector.memset(bias_t[:, :], PBIAS)
            ones_t = inp.tile([128, 1], F16, tag="ones", name="ones_t")
            nc.vector.memset(ones_t[:, :], 1.0)

            # Warm-up prologue: runs while the input DMAs land. ~32 dummy
            # matmuls keep the PE busy >3.4us so the HAM clock-gate opens
            # before the real rounds, and one dummy exp pulls in the ACT
            # table load (~2.7us) that would otherwise stall round 0.
            wsrc = inp.tile([128, 128], F16, tag="wsrc", name="wsrc")
            wjunk = inp.tile([128, 512], F16, tag="wjunk", name="wjunk")
            nc.vector.memset(wsrc[:, :], 0.01)
            nc.vector.memset(wjunk[:, :], 0.01)
            warm = scp.tile([128, 1024], F32, tag="score", name="warm")
            for i in range(32):
                nc.tensor.matmul(warm[:, (i % 2) * 512:(i % 2 + 1) * 512],
                                 wsrc[:, :], wjunk[:, :],
                                 start=(i < 2), stop=(i >= 30))
            wp = ptp.tile([128, 1024], F16, tag="p1", name="warmp")
            nc.scalar.activation(
                wp[:, :512], warm[:, :512],
                mybir.ActivationFunctionType.Exp, scale=ESC, bias=bias_t[:, :])

            qh_sb, k_sb, vh_sb, vl_sb = [], [], [], []
            for s in range(NSEG):
                qh = inp.tile([128, SEG], F16, tag=f"qh{s}", name=f"qh{s}")
                kk = inp.tile([128, SEG], F16, tag=f"k{s}", name=f"k{s}")
                vh = inp.tile([128, NKB * 64], F16, tag=f"vh{s}", name=f"vh{s}")
                vl = inp.tile([128, NKB * 64], F16, tag=f"vl{s}", name=f"vl{s}")
                vsl = slice(s * NKB * 64, (s + 1) * NKB * 64)
                # split the first segment's Q/K transfers across DMA queues so
                # round 0 isn't gated on a single ~512KB queue transfer
                nsl_dma = 4 if s == 0 else 1
                for t, ap_ in ((qh, qhh_ap), (kk, khl_ap)):
                    step = SEG // nsl_dma
                    for z in range(nsl_dma):
                        lo = z * step
                        nc.sync.dma_start(
                            t[:, lo:lo + step],
                            ap_[:, s * SEG + lo:s * SEG + lo + step])
                nc.sync.dma_start(vh[:, :], v1h_ap[:, vsl])
                nc.sync.dma_start(vl[:, :], v1l_ap[:, vsl])
                qh_sb.append(qh)
                k_sb.append(kk)
                vh_sb.append(vh)
                vl_sb.append(vl)

            ot_tiles = {}
            den_tiles = {}
            pend1, pend2 = [], []  # PV work lagged by 1 and 2 steps

            def flush(items):
                for p1ref, t in items:
                    pp, kb = divmod(t, NKB)
                    s = pp // 2
                    if kb == 0:
                        ot_tiles[pp] = otp.tile([128, 512], F32, tag="ot",
                                                name=f"ot{pp}")
                        den_tiles[pp] = denp.tile([65, 512], F32, tag="den",
                                                  name=f"den{pp}")
                    vsl = slice(kb * 64, (kb + 1) * 64)
                    ot = ot_tiles[pp]
                    den = den_tiles[pp]
                    pA = p1ref[:, 0:512]
                    pB = p1ref[:, 512:1024]
                    first, last = kb == 0, kb == NKB - 1
                    nc.tensor.matmul(ot[0:64, :], vh_sb[s][:, vsl], pA,
                                     start=first, stop=False)
                    nc.tensor.matmul(ot[64:128, :], vh_sb[s][:, vsl], pB,
                                     start=first, stop=False)
                    nc.tensor.matmul(ot[0:64, :], vl_sb[s][:, vsl], pA,
                                     start=False, stop=last)
                    nc.tensor.matmul(ot[64:128, :], vl_sb[s][:, vsl], pB,
                                     start=False, stop=last)
                    nc.tensor.matmul(den[0:1, :], ones_t[:, :], pA,
                                     start=first, stop=last)
                    nc.tensor.matmul(den[64:65, :], ones_t[:, :], pB,
                                     start=first, stop=last)
                    if last:
                        osl = slice(pp * 512, (pp + 1) * 512)
                        o_sb = osbp.tile([128, 512], F32, tag="osb",
                                         name=f"osb{pp}")
                        nc.vector.tensor_copy(o_sb[:, :], ot[:, :])
                        nc.sync.dma_start(out2_ap[:, osl], o_sb[:, :])
                        d_sb = osbp.tile([65, 512], F32, tag="dsb",
                                         name=f"dsb{pp}")
                        nc.vector.tensor_copy(d_sb[:, :], den[:, :])
                        nc.sync.dma_start(outd_ap[0:1, osl], d_sb[0:1, :])
                        nc.sync.dma_start(outd_ap[1:2, osl], d_sb[64:65, :])

            for t in range(NSTEP):
                pp, kb = divmod(t, NKB)
                s = pp // 2
                cA, cB = (2 * pp) % 4, (2 * pp + 1) % 4
                score = scp.tile([128, 1024], F32, tag="score",
                                 name=f"score{t}")
                lhsT = k_sb[s][:, kb * 128:(kb + 1) * 128]
                nc.tensor.matmul(score[:, 0:512], lhsT,
                                 qh_sb[s][:, cA * 512:(cA + 1) * 512],
                                 start=True, stop=True)
                nc.tensor.matmul(score[:, 512:1024], lhsT,
                                 qh_sb[s][:, cB * 512:(cB + 1) * 512],
                                 start=True, stop=True)
                p1 = ptp.tile([128, 1024], F16, tag="p1", name=f"p1_{t}")
                nc.scalar.activation(
                    p1[:, :], score[:, :],
                    mybir.ActivationFunctionType.Exp, scale=ESC,
                    bias=bias_t[:, :])
                if t < 2:
                    # startup filler: first PV work arrives after the lag-2
                    # scores->exp pipeline; keep the PE streaming with dummies
                    # aimed at an OT-pool slot (idle until step 2).
                    fill = otp.tile([128, 512], F32, tag="ot", name=f"fill{t}")
                    for z in range(3):
                        nc.tensor.matmul(fill[:, :], wsrc[:, :], wjunk[:, :],
                                         start=(z == 0), stop=(z == 2))
                flush(pend2)
                pend2 = pend1
                pend1 = [(p1, t)]
            flush(pend2)
            flush(pend1)

    nc.compile()
    return nc


def _prep_core(query, key, value, core):
    b, j = divmod(core, 4)
    segs = []
    for arr in (query, key, value):
        h0 = arr[b, :, j, :].reshape(4, SEG, D)
        h1 = arr[b, :, 4 + j, :].reshape(2, 4096, D)[:, 1::2, :]
        h2 = arr[b, 2::4, 8 + j, :][None]
        segs.append(np.concatenate([h0, h1, h2], axis=0))  # [7, 2048, 64]
    qs, ks, vs = segs
    # [64, NSEG*SEG] with col = s*SEG + p
    qt = (qs * QSC).transpose(2, 0, 1).reshape(D, NSEG * SEG)
    kt = (ks * QSC).transpose(2, 0, 1).reshape(D, NSEG * SEG)
    qh = qt.astype(np.float16)
    kh = kt.astype(np.float16)
    kl = (kt - kh).astype(np.float16)
    vv = vs * QSC  # [7, 2048, 64], pre-scaled
    v1 = vv.reshape(NSEG, NKB, 128, 64).transpose(2, 0, 1, 3).reshape(128, -1)
    v1h = v1.astype(np.float16)
    v1l = (v1 - v1h).astype(np.float16)
    return {
        "qhh": np.ascontiguousarray(np.concatenate([qh, qh], axis=0)),
        "khl": np.ascontiguousarray(np.concatenate([kh, kl], axis=0)),
        "v1h": np.ascontiguousarray(v1h),
        "v1l": np.ascontiguousarray(v1l),
    }


def _unshard(results, dtype):
    full = np.zeros((B, N, H, D), dtype)
    for core in range(8):
        b, j = divmod(core, 4)
        o2 = results[core]["out2"].astype(np.float64)
        od = results[core]["outd"].astype(np.float64)
        T = np.empty((64, NCHUNK * 512))
        for pp in range(NPAIR):
            sl = slice(pp * 512, (pp + 1) * 512)
            slA = slice((2 * pp) * 512, (2 * pp + 1) * 512)
            slB = slice((2 * pp + 1) * 512, (2 * pp + 2) * 512)
            T[:, slA] = o2[0:64, sl] / od[0:1, sl]
            T[:, slB] = o2[64:128, sl] / od[1:2, sl]
        h0 = T[:, :4 * SEG]
        full[b, :, j, :] = (h0 / (3.0 * h0.sum(1, keepdims=True))).T
        h1 = T[:, 4 * SEG:6 * SEG]
        h1 = h1 / (3.0 * h1.sum(1, keepdims=True))
        for g in range(2):
            full[b, g * 4096 + 1:(g + 1) * 4096:2, 4 + j, :] = \
                h1[:, g * SEG:(g + 1) * SEG].T
        h2 = T[:, 6 * SEG:]
        full[b, 2::4, 8 + j, :] = (h2 / (3.0 * h2.sum(1, keepdims=True))).T
    return full


def _ensure_axon_backend():
    """The bass PJRT path needs the axon/neuron jax backend. A harness may
    pin JAX_PLATFORMS=cpu for its reference; re-select axon if so."""
    import jax
    try:
        plat = jax.devices()[0].platform
    except Exception:
        plat = ""
    if plat not in ("axon", "neuron"):
        try:
            jax.config.update("jax_platforms", "axon,cpu")
            jax.devices()
        except Exception:
            pass


def kernel(query, key, value):
    _ensure_axon_backend()
    query = np.asarray(query, np.float32)
    key = np.asarray(key, np.float32)
    value = np.asarray(value, np.float32)
    assert query.shape == (B, N, H, D)

    if "nc" not in _CACHE:
        _CACHE["nc"] = _build_nc()
    nc = _CACHE["nc"]

    in_maps = [_prep_core(query, key, value, c) for c in range(8)]
    res = run_bass_kernel_spmd(nc, in_maps, core_ids=list(range(8)))
    LAST_RESULT["exec_time_ns"] = res.exec_time_ns
    return _unshard(res.results, query.dtype)


# revision 5
# speedup vs baseline: 1.4354x; 1.4354x over previous
"""Dilated attention (LongNet-style) Trainium2 kernel — v4 (fused PV).

Problem: query/key/value (2, 8192, 12, 64) f32. Three dilation groups
(segment lengths 2048/4096/8192, dilation 1/2/4, head slices 0:4/4:8/8:12).
Each group's gather produces independent dense attention over 2048-position
dilated segments; outputs are normalized per (batch, head, channel) by the
sum over all segment positions, and divided by num_groups.

Sharding: 8 cores = 2 batches x 4 "head columns". Core c owns batch c//4 and
heads {j, 4+j, 8+j} where j = c%4 -- exactly 7 dense 2048x2048x64 attention
units per core (4 + 2 + 1 segments), perfectly balanced, with all segments of
any (batch, head) on one core so normalization needs no cross-core traffic.

Precision (validated by numpy simulation of the exact arithmetic): the
x / x.sum normalization amplifies V-path errors ~140x but score/P-path
errors only ~8-15x. So K keeps an fp16 hi/lo pair packed along the
contraction dim (khl rows 0-63 = kh, 64-127 = kl; qhh rows = qh duplicated,
so scores = (kh+kl)^T qh in ONE fp16 matmul), Q and P are single fp16, and
V keeps an fp16 hi/lo pair.

v4 trick: a matmul's cost is its N (moving columns), not M, so the PV pair
+ denominator fuse into ONE matmul by packing the stationary operand as
  lhsT = [vh(ch 0-63) | vl(ch 0-62) | ones] (128 x 128):
output rows 0-63 = p1@vh, rows 64-126 = p1@vl, row 127 = softmax denom.
The HOST adds the hi/lo halves in f64. Channel 63 loses its lo-correction
(+~3e-3 error, channel-diagonal). Per 128x512 unit the PE now does just
2 matmuls (scores + fused PV) = 1296ns/round of 3, making the single ACT
exp pass (1530ns/round) the bottleneck. Sim end-to-end: ~6.6e-3 (thr 2e-2).

Device kernel (same program on all 8 cores, different data):
  - inputs: qhh/khl [128, 14336] fp16 (as above), vhl [128, 14336] fp16
    (the packed 128x128 stationary blocks per (seg, k-block)).
  - per (chunk, k-block) unit (28 q-chunks of 512 x 16 k-blocks):
      S^T = khl_blk.T @ qhh              (PE, 1 MM, PSUM f32)
      p1 = fp16(exp(S^T*0.125/65536 + ln64))   (ACT, PSUM -> SBUF)
      O'[128, 512] += vhl_blk.T @ p1     (PE, 1 MM, f32 PSUM, accum over kb)
  - O' copied PSUM->SBUF (DVE) into a per-segment staging tile, DMA'd to
    DRAM out [128, 14336] f32 once per segment (4 chunks).
Host: num = O'[0:64] (+= O'[64:127] for ch<63), den = O'[127], T = num/den,
then the group normalization (sum over positions) and /3, scattered into
the (2, 8192, 12, 64) output. Positions not in a dilated group stay zero.
"""

import os
import sys

if "/opt/trn_rl_repo" not in sys.path:
    sys.path.insert(0, "/opt/trn_rl_repo")
if "jax" not in sys.modules:
    os.environ.setdefault("JAX_PLATFORMS", "axon")

import numpy as np

import concourse.bass as bass  # noqa: F401
import concourse.mybir as mybir
import concourse.tile as tile
from concourse import bacc
from concourse.bass_utils import run_bass_kernel_spmd

F32 = mybir.dt.float32
F16 = mybir.dt.float16

B, N, H, D = 2, 8192, 12, 64
NSEG = 7           # segments per core
SEG = 2048         # dilated segment length
NCHUNK = NSEG * 4  # 512-wide q chunks per core
NKB = 16           # 128-row k blocks per segment
NUNIT = NCHUNK * NKB
RW = 3             # k-blocks per exp round (3 PSUM banks per ACT span)
QSC = np.float32(256.0)               # fp16 pre-scale for Q/K/V splits
ESC = float(0.125 / (256.0 * 256.0))  # exp scale: 1/sqrt(64) + descale
import math
PBIAS = float(math.log(64.0))         # exp bias: P *= 64, into fp16-normal range

_CACHE = {}
LAST_RESULT = {}


def _build_nc():
    nc = bacc.Bacc("TRN2", target_bir_lowering=False, debug=False,
                   enable_asserts=False, num_devices=8)
    qhh = nc.dram_tensor("qhh", [128, NSEG * SEG], F16, kind="ExternalInput")
    khl = nc.dram_tensor("khl", [128, NSEG * SEG], F16, kind="ExternalInput")
    vhl = nc.dram_tensor("vhl", [128, NSEG * NKB * 128], F16,
                         kind="ExternalInput")
    out = nc.dram_tensor("out", [128, NCHUNK * 512], F32, kind="ExternalOutput")
    qhh_ap, khl_ap, vhl_ap, out_ap = qhh.ap(), khl.ap(), vhl.ap(), out.ap()

    with tile.TileContext(nc) as tc:
        with (
            tc.tile_pool(name="inp", bufs=1) as inp,
            tc.tile_pool(name="pt", bufs=4) as ptp,
            tc.tile_pool(name="osb", bufs=2) as osbp,
            tc.tile_pool(name="score", bufs=2, space="PSUM") as scp,
            tc.tile_pool(name="ot", bufs=2, space="PSUM") as otp,
        ):
            bias_t = inp.tile([128, 1], F32, tag="bias", name="bias_t")
            nc.vector.memset(bias_t[:, :], PBIAS)

            # Minimal warm-up: 3 dummy matmuls complete a PSUM slice fast so
            # the dummy exp (and with it the ~2.7us ACT table load) fires
            # early, overlapping the first input DMAs. The HAM clock-gate
            # opens during the first real rounds (ACT-bound pipeline absorbs
            # the short cold-PE ramp).
            wsrc = inp.tile([128, 128], F16, tag="wsrc", name="wsrc")
            wjunk = inp.tile([128, 512], F16, tag="wjunk", name="wjunk")
            nc.vector.memset(wsrc[:, :], 0.01)
            nc.vector.memset(wjunk[:, :], 0.01)
            warm = scp.tile([128, 512 * RW], F32, tag="score", name="warm")
            for i in range(3):
                nc.tensor.matmul(warm[:, :512], wsrc[:, :], wjunk[:, :],
                                 start=(i == 0), stop=(i == 2))
            wp = ptp.tile([128, 512 * RW], F16, tag="p1", name="warmp")
            nc.scalar.activation(
                wp[:, :512], warm[:, :512],
                mybir.ActivationFunctionType.Exp, scale=ESC, bias=bias_t[:, :])

            qh_sb, k_sb, v_sb = [], [], []
            for s in range(NSEG):
                qh = inp.tile([128, SEG], F16, tag=f"qh{s}", name=f"qh{s}")
                kk = inp.tile([128, SEG], F16, tag=f"k{s}", name=f"k{s}")
                vv = inp.tile([128, NKB * 128], F16, tag=f"v{s}", name=f"v{s}")
                # split the first segment's transfers across DMA queues so
                # round 0 isn't gated on a single ~512KB queue transfer
                nsl_dma = 4 if s == 0 else 1
                for t, ap_, w in ((qh, qhh_ap, SEG), (kk, khl_ap, SEG),
                                  (vv, vhl_ap, NKB * 128)):
                    step = w // nsl_dma
                    for z in range(nsl_dma):
                        lo = z * step
                        nc.sync.dma_start(
                            t[:, lo:lo + step],
                            ap_[:, s * w + lo:s * w + lo + step])
                qh_sb.append(qh)
                k_sb.append(kk)
                v_sb.append(vv)

            ot_tiles = {}
            oseg_tiles = {}
            pend1, pend2 = [], []  # PV work lagged by 1 and 2 rounds

            def flush(items):
                for p1ref, i, u in items:
                    cid, kb = divmod(u, NKB)
                    s, c = divmod(cid, 4)
                    if kb == 0:
                        ot_tiles[cid] = otp.tile([128, 512], F32, tag="ot",
                                                 name=f"ot{cid}")
                    vsl = slice(kb * 128, (kb + 1) * 128)
                    psl = slice(i * 512, (i + 1) * 512)
                    nc.tensor.matmul(ot_tiles[cid][:, :], v_sb[s][:, vsl],
                                     p1ref[:, psl],
                                     start=(kb == 0), stop=(kb == NKB - 1))
                    if kb == NKB - 1:
                        if c == 0:
                            oseg_tiles[s] = osbp.tile(
                                [128, 4 * 512], F32, tag="oseg",
                                name=f"oseg{s}")
                        osl = slice(c * 512, (c + 1) * 512)
                        nc.vector.tensor_copy(oseg_tiles[s][:, osl],
                                              ot_tiles[cid][:, :])
                        if c == 3:
                            nc.sync.dma_start(
                                out_ap[:, s * 2048:(s + 1) * 2048],
                                oseg_tiles[s][:, :])

            for r in range((NUNIT + RW - 1) // RW):
                units = range(r * RW, min((r + 1) * RW, NUNIT))
                nu = len(units)
                score = scp.tile([128, 512 * RW], F32, tag="score",
                                 name=f"score{r}")
                for i, u in enumerate(units):
                    cid, kb = divmod(u, NKB)
                    s, c = divmod(cid, 4)
                    osl = slice(i * 512, (i + 1) * 512)
                    csl = slice(c * 512, (c + 1) * 512)
                    lhsT = k_sb[s][:, kb * 128:(kb + 1) * 128]
                    nc.tensor.matmul(score[:, osl], lhsT, qh_sb[s][:, csl],
                                     start=True, stop=True)
                nsl = slice(0, 512 * nu)
                p1 = ptp.tile([128, 512 * RW], F16, tag="p1", name=f"p1_{r}")
                nc.scalar.activation(
                    p1[:, nsl], score[:, nsl],
                    mybir.ActivationFunctionType.Exp, scale=ESC,
                    bias=bias_t[:, :])
                if r < 2:
                    # startup filler: PV work arrives only after the lag-2
                    # scores->exp pipeline; keep the PE from a long idle
                    # (HAM) with dummies aimed at an OT-pool slot.
                    fill = otp.tile([128, 512], F32, tag="ot", name=f"fill{r}")
                    for z in range(3):
                        nc.tensor.matmul(fill[:, :], wsrc[:, :], wjunk[:, :],
                                         start=(z == 0), stop=(z == 2))
                flush(pend2)
                pend2 = pend1
                pend1 = [(p1, i, u) for i, u in enumerate(units)]
            flush(pend2)
            flush(pend1)

    nc.compile()
    return nc


def _prep_core(query, key, value, core):
    b, j = divmod(core, 4)
    segs = []
    for arr in (query, key, value):
        h0 = arr[b, :, j, :].reshape(4, SEG, D)
        h1 = arr[b, :, 4 + j, :].reshape(2, 4096, D)[:, 1::2, :]
        h2 = arr[b, 2::4, 8 + j, :][None]
        segs.append(np.concatenate([h0, h1, h2], axis=0))  # [7, 2048, 64]
    qs, ks, vs = segs
    # [64, NSEG*SEG] with col = s*SEG + p
    qt = (qs * QSC).transpose(2, 0, 1).reshape(D, NSEG * SEG)
    kt = (ks * QSC).transpose(2, 0, 1).reshape(D, NSEG * SEG)
    qh = qt.astype(np.float16)
    kh = kt.astype(np.float16)
    kl = (kt - kh).astype(np.float16)
    vv = vs * QSC  # [7, 2048, 64] f32, pre-scaled
    v1h = vv.astype(np.float16)
    v1l = (vv - v1h).astype(np.float16)
    # packed stationary blocks: [vh(64) | vl(ch 0-62) | ones] per k-block
    blk = np.empty((NSEG, SEG, 128), np.float16)
    blk[:, :, 0:64] = v1h
    blk[:, :, 64:127] = v1l[:, :, 0:63]
    blk[:, :, 127] = np.float16(256.0)
    vhl = blk.reshape(NSEG, NKB, 128, 128).transpose(2, 0, 1, 3).reshape(128, -1)
    return {
        "qhh": np.ascontiguousarray(np.concatenate([qh, qh], axis=0)),
        "khl": np.ascontiguousarray(np.concatenate([kh, kl], axis=0)),
        "vhl": np.ascontiguousarray(vhl),
    }


def _unshard(results, dtype):
    full = np.zeros((B, N, H, D), dtype)
    for core in range(8):
        b, j = divmod(core, 4)
        o = results[core]["out"].astype(np.float64)
        num = o[0:64].copy()
        num[0:63] += o[64:127]
        T = num / o[127:128]  # [64, 14336]
        h0 = T[:, :4 * SEG]
        full[b, :, j, :] = (h0 / (3.0 * h0.sum(1, keepdims=True))).T
        h1 = T[:, 4 * SEG:6 * SEG]
        h1 = h1 / (3.0 * h1.sum(1, keepdims=True))
        for g in range(2):
            full[b, g * 4096 + 1:(g + 1) * 4096:2, 4 + j, :] = \
                h1[:, g * SEG:(g + 1) * SEG].T
        h2 = T[:, 6 * SEG:]
        full[b, 2::4, 8 + j, :] = (h2 / (3.0 * h2.sum(1, keepdims=True))).T
    return full


def _ensure_axon_backend():
    """The bass PJRT path needs the axon/neuron jax backend. A harness may
    pin JAX_PLATFORMS=cpu for its reference; re-select axon if so."""
    import jax
    try:
        plat = jax.devices()[0].platform
    except Exception:
        plat = ""
    if plat not in ("axon", "neuron"):
        try:
            jax.config.update("jax_platforms", "axon,cpu")
            jax.devices()
        except Exception:
            pass


def kernel(query, key, value):
    _ensure_axon_backend()
    query = np.asarray(query, np.float32)
    key = np.asarray(key, np.float32)
    value = np.asarray(value, np.float32)
    assert query.shape == (B, N, H, D)

    if "nc" not in _CACHE:
        _CACHE["nc"] = _build_nc()
    nc = _CACHE["nc"]

    in_maps = [_prep_core(query, key, value, c) for c in range(8)]
    res = run_bass_kernel_spmd(nc, in_maps, core_ids=list(range(8)))
    LAST_RESULT["exec_time_ns"] = res.exec_time_ns
    return _unshard(res.results, query.dtype)
